# revision 31
# baseline (speedup 1.0000x reference)
"""Trainium2 Bass kernel for the DGL-style heterogeneous temporal GNN.

Model (per reference):
  for t in 0..T-1:   h1 = relu(sum_r GraphConv_r(feat[t]));  h2 = relu(sum_r GraphConv_r(h1))
  h_last = GRU over t of h2;  logits = MLP(h_last)

GraphConv_r(x)[d] = (sum_{e: dst_e=d} w_e * x[src_e]) / max(indeg_r(d),1) @ W_r + b_r

Distribution: 1D node partition over 8 NeuronCores.  Each core owns N/8
destination nodes and processes every edge pointing into its shard.  feat is
uploaded sharded and AllGathered on device into the padded-global layout
[NC*NSP, D]; h1 shards are likewise AllGathered so conv2 can gather
arbitrary source rows.  Both conv layers share one set of gather indices /
edge scales / scatter destinations per timestep.  GRU/MLP are data-parallel
over the shard.

Edge aggregation on-core via the SWDGE batched custom DMA ops:
  - sources are binned into 4 contiguous table ranges of 25600 rows so
    dma_gather's int16 indices can address them;
  - per (range, size-class) the host packs each destination's edges into a
    fixed-size class-G window slot grid (col-major: slot s -> partition
    s%128, column s//128 to match the ucode's index<->data pairing);
  - per tile the kernel does ONE dma_gather of all [128, k] slot rows,
    multiplies by w/deg on DVE, window-reduces on DVE, and ONE
    dma_scatter_add per relation segment accumulates the group rows into
    the pre-zeroed per-relation aggregation buffer.
"""
import sys
import time as _time

sys.path.insert(0, "/opt/trn_rl_repo")
import numpy as np

import os as _os

TRACE = False
LAST_EXEC_NS = None
VERBOSE = _os.environ.get("KERNEL_VERBOSE", "") == "1"


def _log(msg):
    if VERBOSE:
        print(f"[kernel] {msg}", file=sys.stderr, flush=True)


CFG = dict(
    T=4, R=3, N=100000, E=800000, D=64,
    NCORES=8,
    NSP=12800,            # padded shard rows (must be mult of 512)
    KTILE=96,             # max slot columns per gather tile (num_idxs=128*k
                          # must stay under the 64KB Q7 scratch: k <= ~127)
    NRANGE=4,             # source-table ranges (rows/range = NC*NSP/NRANGE)
    GCLASSES=(1, 2, 3, 4, 5, 6, 8, 10, 12, 16, 24, 32, 64),
)


def _segment_arange(sizes):
    """[3,2] -> [0,1,2,0,1]"""
    total = int(sizes.sum())
    if total == 0:
        return np.zeros(0, np.int64)
    starts = np.cumsum(sizes) - sizes
    return np.arange(total) - np.repeat(starts, sizes)


def _wrap16(flat):
    """[n] -> [32, n//16] ucode idx layout: arr[i%16, i//16] = flat[i],
    replicated into partitions 16-31 (the tx Q7 core's partition group)."""
    assert flat.size % 16 == 0
    w = flat.reshape(-1, 16).T
    return np.ascontiguousarray(np.vstack([w, w]))


# --------------------------------------------------------------------------
# host-side preprocessing
# --------------------------------------------------------------------------

def preprocess(src, dst, ew, cfg=None):
    """Build per-core edge streams (shared by conv1 and conv2).

    Returns (per_core, meta).  per_core[c] holds, per t:
      gidx_t [16, 8*CK] i16  gather indices (wrapped), slot s=col*128+p
      sca_t  [128, CK]  f32  per-slot edge scale
      sidx_t [16, 8*CQ] i16  scatter dst locs (wrapped), group n=col*128+p
    meta['tiles'][t] = list of dicts(stream, G, k, q, segs=[(r, cq0, cq1)]).
    """
    cfg = cfg or CFG
    T, R, N, E, D = cfg["T"], cfg["R"], cfg["N"], cfg["E"], cfg["D"]
    NC, NSP = cfg["NCORES"], cfg["NSP"]
    NS = N // NC
    KT = cfg["KTILE"]
    NRG = cfg["NRANGE"]
    RROWS = NC * NSP // NRG           # rows per source range
    GCL = cfg["GCLASSES"]
    GBIG = GCL[-1]
    DUMMY = NS                        # dummy dst loc inside the pad region

    src = np.asarray(src).astype(np.int64)
    dst = np.asarray(dst).astype(np.int64)
    ew = np.asarray(ew).astype(np.float32)

    meta = {"tiles": {t: [] for t in range(T)}, "cfg": dict(cfg)}
    per_core = [dict() for _ in range(NC)]

    rel_id = np.repeat(np.arange(R, dtype=np.int64), E)
    gcl_arr = np.asarray(GCL)

    for t in range(T):
        # per-relation degree over the full graph + normalized weights
        wt_t = np.empty((R, E), np.float32)
        for r in range(R):
            deg = np.bincount(dst[t, r], minlength=N)
            wt_t[r] = ew[t, r] / np.maximum(deg[dst[t, r]], 1)

        dstf = dst[t].reshape(-1)
        srcf = src[t].reshape(-1)
        wtf = wt_t.reshape(-1)
        c_id = dstf // NS
        loc = dstf - c_id * NS
        gi_glob = (srcf // NS) * NSP + srcf % NS
        stream = gi_glob // RROWS
        lidx = (gi_glob - stream * RROWS).astype(np.int16)
        # sort by (core, stream, r, loc)
        key = (((c_id * NRG + stream) * R + rel_id) * NSP + loc).astype(
            np.int32)
        order = np.argsort(key)
        key_s = key[order]
        lidx_s = lidx[order]
        gw_s = wtf[order].astype(np.float32)

        # boundaries of (core, stream) buckets
        cs_bnd = np.searchsorted(
            key_s, (np.arange(NC * NRG + 1) * (R * NSP)).astype(np.int32))

        # per (core, stream): group lists per class with per-relation splits
        # blocks[(s, gi)] = per-core list of (loc_arr, size_arr, estart_arr,
        #                                     r_arr)
        blocks = {}
        for c in range(NC):
            for s in range(NRG):
                b = c * NRG + s
                lo, hi = cs_bnd[b], cs_bnd[b + 1]
                sub = key_s[lo:hi] - (b * R) * NSP   # = r*NSP + loc
                if sub.size:
                    bnd = np.flatnonzero(np.diff(sub)) + 1
                    starts = np.concatenate([[0], bnd])
                    uk = sub[starts]
                    counts = np.diff(np.concatenate([starts, [sub.size]]))
                else:
                    uk = np.zeros(0, np.int64)
                    counts = np.zeros(0, np.int64)
                estart = (np.cumsum(counts) - counts) + lo
                # split groups larger than GBIG (scatter-add makes it legal)
                if counts.size and counts.max() > GBIG:
                    reps = -(-counts // GBIG)
                    uk = np.repeat(uk, reps)
                    within = _segment_arange(reps)
                    counts_r = np.repeat(counts, reps)
                    estart = np.repeat(estart, reps) + within * GBIG
                    counts = np.minimum(counts_r - within * GBIG, GBIG)
                g_r = uk // NSP
                g_loc = uk % NSP
                cls = np.searchsorted(gcl_arr, counts)
                for gi_c in range(len(GCL)):
                    m = cls == gi_c
                    blocks.setdefault((s, gi_c), [[] for _ in range(NC)])
                    blocks[(s, gi_c)][c] = (
                        g_loc[m], counts[m], estart[m], g_r[m])

        # build per (stream, class) grids with uniform shape across cores
        gidx_cols = [[] for _ in range(NC)]   # list of [16, 8k] blocks
        sca_cols = [[] for _ in range(NC)]    # list of [128, k] blocks
        sidx_cols = [[] for _ in range(NC)]   # list of [16, 8q] blocks
        for s in range(NRG):
            for gi_c, G in enumerate(GCL):
                if (s, gi_c) not in blocks:
                    continue
                percore = blocks[(s, gi_c)]
                # per-relation padded sizes (uniform across cores, 128-mult)
                ng_r = []
                for r in range(R):
                    mx = max((d[3] == r).sum() if len(d) else 0
                             for d in percore)
                    ng_r.append(-(-int(mx) // 128) * 128)
                NG = sum(ng_r)
                if NG == 0:
                    continue
                Q = NG // 128
                qmax = max(1, KT // G)
                # tile split points (in group-columns)
                tsplit = list(range(0, Q, qmax)) + [Q]
                cq_r = np.cumsum([0] + [n // 128 for n in ng_r])
                for ti in range(len(tsplit) - 1):
                    c0, c1 = tsplit[ti], tsplit[ti + 1]
                    segs = []
                    for r in range(R):
                        s0, s1 = max(c0, cq_r[r]), min(c1, cq_r[r + 1])
                        if s1 > s0:
                            segs.append((r, s0 - c0, s1 - c0))
                    meta["tiles"][t].append(dict(
                        stream=s, G=G, k=(c1 - c0) * G, q=c1 - c0, segs=segs))
                for c in range(NC):
                    d = percore[c]
                    # grid arrays [NG]
                    loc_a = np.full(NG, DUMMY, np.int16)
                    size_a = np.zeros(NG, np.int64)
                    est_a = np.zeros(NG, np.int64)
                    if len(d):
                        gl, gs, ge, gr = d
                        for r in range(R):
                            m = gr == r
                            base = int(cq_r[r]) * 128
                            nn = int(m.sum())
                            loc_a[base:base + nn] = gl[m]
                            size_a[base:base + nn] = gs[m]
                            est_a[base:base + nn] = ge[m]
                    # fill slot grid
                    kblk = Q * G
                    gidx_f = np.zeros(128 * kblk, np.int16)
                    sca_f = np.zeros((128, kblk), np.float32)
                    n_idx = np.arange(NG)
                    p_n = n_idx % 128
                    col_n = n_idx // 128
                    within = _segment_arange(size_a)
                    epos = np.repeat(est_a, size_a) + within
                    sc_col = np.repeat(col_n * G, size_a) + within
                    p_rep = np.repeat(p_n, size_a)
                    sflat = sc_col * 128 + p_rep
                    gidx_f[sflat] = lidx_s[epos]
                    sca_f[p_rep, sc_col] = gw_s[epos]
                    gidx_cols[c].append(_wrap16(gidx_f))
                    sca_cols[c].append(sca_f)
                    sidx_cols[c].append(_wrap16(loc_a))

        for c in range(NC):
            pc = per_core[c]
            pc[f"gidx_{t}"] = np.concatenate(gidx_cols[c], axis=1)
            pc[f"sca_{t}"] = np.concatenate(sca_cols[c], axis=1)
            pc[f"sidx_{t}"] = np.concatenate(sidx_cols[c], axis=1)
    return per_core, meta


def pack_inputs(per_core, meta, feat, wts, cfg=None):
    """Merge per-t arrays into 4 upload tensors per core (the axon tunnel
    has a large per-array fixed cost):
      eidx [32, *] i16   (gidx_t | sidx_t per t, column-concatenated)
      esca [128, CKtot] bf16
      fshb [T*NSP, D] bf16  (cast back to f32 on device)
      wblob [128, WC] f32
    Offsets are recorded in meta.
    """
    import ml_dtypes
    bf16 = ml_dtypes.bfloat16
    cfg = cfg or CFG
    T, N, D = cfg["T"], cfg["N"], cfg["D"]
    NC, NSP = cfg["NCORES"], cfg["NSP"]
    NS = N // NC

    goff, soff, scoff = {}, {}, {}
    col = 0
    scol = 0
    for t in range(T):
        ck = sum(d["k"] for d in meta["tiles"][t])
        cq = sum(d["q"] for d in meta["tiles"][t])
        goff[t] = col
        col += 8 * ck
        soff[t] = col
        col += 8 * cq
        scoff[t] = scol
        scol += ck
    meta["goff"], meta["soff"], meta["scoff"] = goff, soff, scoff
    meta["eidx_cols"], meta["esca_cols"] = col, scol

    # weight blob layout
    wcols = {}
    wc = 0
    for nm, arr in wts.items():
        wcols[nm] = (arr.shape[0], wc, arr.shape[1])
        wc += arr.shape[1]
    meta["wcols"], meta["wblob_cols"] = wcols, wc
    wblob = np.zeros((128, wc), np.float32)
    for nm, arr in wts.items():
        r, c0, w = wcols[nm]
        wblob[:r, c0:c0 + w] = arr

    out = []
    for c in range(NC):
        pc = per_core[c]
        eidx = np.concatenate(
            [x for t in range(T) for x in (pc[f"gidx_{t}"], pc[f"sidx_{t}"])],
            axis=1)
        esca = np.concatenate(
            [pc[f"sca_{t}"] for t in range(T)], axis=1).astype(bf16)
        fshb = np.zeros((T * NSP, D), bf16)
        for t in range(T):
            fshb[t * NSP: t * NSP + NS] = feat[t][c * NS:(c + 1) * NS]
        out.append(dict(eidx=eidx, esca=esca, fshb=fshb, wblob=wblob))
    return out


def make_weight_inputs(W1, b1, W2, b2, Wih, Whh, bih, bhh, Wc1, bc1, Wc2, bc2,
                       cfg=None):
    cfg = cfg or CFG
    D = cfg["D"]
    H = D
    f = np.float32
    out = dict(
        w1s=np.vstack([W1[0], W1[1]]).astype(f),          # [2D, D]
        w1r2=np.asarray(W1[2], f),                        # [D, D]
        w2s=np.vstack([W2[0], W2[1]]).astype(f),
        w2r2=np.asarray(W2[2], f),
        b1b=np.broadcast_to(np.asarray(b1, f).sum(0), (128, D)).copy(),
        b2c=np.asarray(b2, f).sum(0)[:, None].copy(),     # [D, 1]
        wih=np.asarray(Wih, f).T.copy(),                  # [D, 3H]
        whh=np.asarray(Whh, f).T.copy(),
        gbr=(np.asarray(bih, f)[0:H] + np.asarray(bhh, f)[0:H])[:, None].copy(),
        gbz=(np.asarray(bih, f)[H:2*H] + np.asarray(bhh, f)[H:2*H])[:, None].copy(),
        gbin=np.asarray(bih, f)[2*H:3*H][:, None].copy(),
        gbhn=np.asarray(bhh, f)[2*H:3*H][:, None].copy(),
        wc1=np.asarray(Wc1, f),                           # [D, D]
        bc1c=np.asarray(bc1, f)[:, None].copy(),          # [D, 1]
        wc2=np.asarray(Wc2, f),                           # [D, 1]
        bc2c=np.asarray(bc2, f).reshape(1, 1).copy(),
    )
    return out


# --------------------------------------------------------------------------
# device program
# --------------------------------------------------------------------------

def build_program(meta):
    from concourse import bacc, bass, mybir, tile
    from concourse.masks import make_identity

    cfg = meta["cfg"]
    T, R, N, D = cfg["T"], cfg["R"], cfg["N"], cfg["D"]
    NC, NSP = cfg["NCORES"], cfg["NSP"]
    NS = N // NC
    NV2 = NC * NSP
    NRG = cfg["NRANGE"]
    RROWS = NV2 // NRG
    KT = cfg["KTILE"]
    MTILES = NSP // 512
    f32 = mybir.dt.float32
    bf16 = mybir.dt.bfloat16
    i16 = mybir.dt.int16
    AF = mybir.ActivationFunctionType
    ALU = mybir.AluOpType

    nc = bacc.Bacc("TRN2", target_bir_lowering=False, debug=False)

    # ---- dram inputs (4 packed tensors; axon has a per-array fixed cost)
    ck_t, cq_t = {}, {}
    for t in range(T):
        ck_t[t] = sum(d["k"] for d in meta["tiles"][t])
        cq_t[t] = sum(d["q"] for d in meta["tiles"][t])
    eidx_d = nc.dram_tensor("eidx", [32, meta["eidx_cols"]], i16,
                            kind="ExternalInput")
    esca_d = nc.dram_tensor("esca", [128, meta["esca_cols"]], bf16,
                            kind="ExternalInput")
    fshb_d = nc.dram_tensor("fshb", [T * NSP, D], bf16, kind="ExternalInput")
    wblob_d = nc.dram_tensor("wblob", [128, meta["wblob_cols"]], f32,
                             kind="ExternalInput")
    goff, soff, scoff = meta["goff"], meta["soff"], meta["scoff"]
    wcols = meta["wcols"]

    out_d = nc.dram_tensor("out", [1, NS], f32, kind="ExternalOutput")

    # ---- dram internals
    agg_d = [[nc.dram_tensor(f"agg{b}_{r}", [NSP, D], f32) for r in range(R)]
             for b in range(2)]
    ag_in = [nc.dram_tensor(f"agin{t}", [NSP, D], f32) for t in range(T)]
    fin = [nc.dram_tensor(f"fin{t}", [NSP, D], f32) for t in range(T)]
    featf = [nc.dram_tensor(f"featf{t}", [NV2, D], f32, addr_space="Shared")
             for t in range(T)]
    h1f = [nc.dram_tensor(f"h1f{t}", [NV2, D], f32, addr_space="Shared")
           for t in range(T)]
    h2T_d = [nc.dram_tensor(f"h2T{t}", [D, NSP], f32) for t in range(T)]

    CKMAX = max(ck_t.values())
    QMAX = max(max(d["q"] for d in meta["tiles"][t]) for t in range(T))
    QGMAX = max(max((d["q"] for d in meta["tiles"][t] if d["G"] > 1),
                    default=1) for t in range(T))

    with tile.TileContext(nc) as tc:
        with tc.tile_pool(name="const", bufs=1) as cpool:
            wtile = cpool.tile([128, meta["wblob_cols"]], f32, tag="wtile")
            nc.sync.dma_start(wtile[:], wblob_d[:])
            wt = {nm: wtile[0:r, c0:c0 + w]
                  for nm, (r, c0, w) in wcols.items()}
            ident = cpool.tile([128, 128], f32, tag="ident")
            make_identity(nc, ident[:])
            czero = cpool.tile([128, 25, 64], f32, tag="czero")
            nc.vector.memset(czero[:], 0.0)

            nreg = {}

            def nidx_reg(n):
                if n not in nreg:
                    nreg[n] = nc.gpsimd.to_reg(n)
                return nreg[n]

            def edge_phase(t, src_full, aggs):
                """zero agg; gather -> scale -> window-reduce -> scatter-add."""
                ck = ck_t[t]
                scab = epool.tile([128, CKMAX], bf16, tag="scab")
                nc.scalar.dma_start(
                    scab[:, 0:ck], esca_d[:, scoff[t]:scoff[t] + ck])
                sca = epool.tile([128, CKMAX], f32, tag="sca")
                nc.vector.tensor_copy(sca[:, 0:ck], scab[:, 0:ck])
                for r in range(R):
                    for j in range(4):
                        nc.sync.dma_start(
                            aggs[r][3200 * j: 3200 * (j + 1)]
                            .rearrange("(j p) d -> p j d", p=128),
                            czero[:])
                tabs = [src_full[RROWS * s: RROWS * (s + 1)]
                        for s in range(NRG)]
                kofs = 0
                qofs = 0
                for td in meta["tiles"][t]:
                    G, k, q, s = td["G"], td["k"], td["q"], td["stream"]
                    gidx = epool.tile([32, 8 * KT], i16, tag="gidx")
                    nc.scalar.dma_start(
                        gidx[:, 0:8 * k],
                        eidx_d[:, goff[t] + 8 * kofs: goff[t] + 8 * (kofs + k)])
                    sidx = epool.tile([32, 8 * QMAX], i16, tag="sidx")
                    nc.scalar.dma_start(
                        sidx[:, 0:8 * q],
                        eidx_d[:, soff[t] + 8 * qofs: soff[t] + 8 * (qofs + q)])
                    msgs = epool.tile([128, KT, D], f32, tag="msgs")
                    # the SWDGE descriptor ring holds 1024 descs; split the
                    # gather into <=8-column (1024-index) calls
                    for j0 in range(0, k, 8):
                        j1 = min(j0 + 8, k)
                        nc.gpsimd.dma_gather(
                            msgs[:, j0:j1, :], tabs[s],
                            gidx[:, 8 * j0: 8 * j1],
                            128 * (j1 - j0), nidx_reg(128 * (j1 - j0)), D)
                    nc.vector.tensor_tensor(
                        out=msgs[:, 0:k, :], in0=msgs[:, 0:k, :],
                        in1=sca[:, kofs:kofs + k, None].to_broadcast(
                            [128, k, D]),
                        op=ALU.mult,
                    )
                    if G == 1:
                        grp_ap = msgs
                    else:
                        grp = epool.tile([128, QGMAX, D], f32, tag="grp")
                        nc.vector.tensor_reduce(
                            out=grp[:, 0:q, :],
                            in_=msgs[:, 0:k, :].rearrange(
                                "p (q g) d -> p q d g", g=G),
                            axis=mybir.AxisListType.X, op=ALU.add,
                        )
                        grp_ap = grp
                    for (r, cq0, cq1) in td["segs"]:
                        for c0 in range(cq0, cq1, 8):
                            c1 = min(c0 + 8, cq1)
                            nc.gpsimd.dma_scatter_add(
                                aggs[r][:], grp_ap[:, c0:c1, :],
                                sidx[:, 8 * c0: 8 * c1],
                                128 * (c1 - c0), nidx_reg(128 * (c1 - c0)), D)
                    kofs += k
                    qofs += q

            def dense_phase(t, layer, aggs):
                """agg -> (conv matmuls + bias + relu) -> h1 shard / h2T."""
                for m in range(MTILES):
                    rows = slice(512 * m, 512 * (m + 1))
                    a01 = pool.tile([128, 4, 2, D], f32, tag="a01")
                    a2 = pool.tile([128, 4, D], f32, tag="a2")
                    for r in range(2):
                        nc.sync.dma_start(
                            a01[:, :, r, :],
                            aggs[r][rows].rearrange("(j p) d -> p j d", p=128))
                    nc.sync.dma_start(
                        a2[:],
                        aggs[2][rows].rearrange("(j p) d -> p j d", p=128))
                    # transposes: a01 block j -> psum[128, 128] (r0 | r1 feats)
                    ps01 = psum.tile([128, 4, 128], f32, tag="ps01", space="PSUM")
                    for j in range(4):
                        nc.tensor.transpose(
                            ps01[:, j, :],
                            a01[:, j, :, :].rearrange("p a d -> p (a d)"),
                            ident[:])
                    aT01 = pool.tile([128, 4, 128], f32, tag="aT01")
                    nc.vector.tensor_copy(aT01[:], ps01[:])
                    ps2 = psum.tile([64, 4, 128], f32, tag="ps2", space="PSUM")
                    for j in range(4):
                        nc.tensor.transpose(
                            ps2[:, j, :], a2[:, j, :], ident[:])
                    aT2 = pool.tile([64, 4, 128], f32, tag="aT2")
                    nc.vector.tensor_copy(aT2[:], ps2[:])

                    if layer == 1:
                        po = psum.tile([128, 4, D], f32, tag="po_nm", space="PSUM")
                        for j in range(4):
                            nc.tensor.matmul(po[:, j, :], aT01[:, j, :],
                                             wt["w1s"][:], start=True, stop=False)
                            nc.tensor.matmul(
                                po[:, j, :], aT2[:, j, :],
                                wt["w1r2"][:], start=False, stop=True)
                        hb = pool.tile([128, 4, D], f32, tag="hb")
                        nc.vector.tensor_tensor(
                            out=hb[:], in0=po[:],
                            in1=wt["b1b"][:, None, :].to_broadcast([128, 4, D]),
                            op=ALU.add)
                        h1t = pool.tile([128, 4, D], f32, tag="h1t")
                        nc.scalar.activation(h1t[:], hb[:], AF.Relu)
                        nc.sync.dma_start(
                            ag_in[t][rows].rearrange("(j p) d -> p j d", p=128),
                            h1t[:])
                    else:
                        po = psum.tile([D, 4, 128], f32, tag="po_fm", space="PSUM")
                        for j in range(4):
                            nc.tensor.matmul(po[:, j, :], wt["w2s"][:],
                                             aT01[:, j, :], start=True, stop=False)
                            nc.tensor.matmul(
                                po[:, j, :], wt["w2r2"][:], aT2[:, j, :],
                                start=False, stop=True)
                        h2t = pool.tile([D, 4, 128], f32, tag="h2t")
                        nc.scalar.activation(h2t[:], po[:], AF.Relu,
                                             bias=wt["b2c"][:])
                        nc.sync.dma_start(h2T_d[t][:, rows], h2t[:])

            with (
                tc.tile_pool(name="edge", bufs=2) as epool,
                tc.tile_pool(name="work", bufs=3) as pool,
                tc.tile_pool(name="ps", bufs=2, space="PSUM") as psum,
            ):
                for t in range(T):
                    for j in range(4):
                        rows = slice(t * NSP + 3200 * j,
                                     t * NSP + 3200 * (j + 1))
                        frows = slice(3200 * j, 3200 * (j + 1))
                        fb = epool.tile([128, 25, D], bf16, tag="fb")
                        nc.sync.dma_start(
                            fb[:], fshb_d[rows]
                            .rearrange("(j p) d -> p j d", p=128))
                        ff = epool.tile([128, 25, D], f32, tag="ff")
                        nc.vector.tensor_copy(ff[:], fb[:])
                        nc.sync.dma_start(
                            fin[t][frows].rearrange("(j p) d -> p j d", p=128),
                            ff[:])
                    nc.gpsimd.collective_compute(
                        "AllGather", ALU.bypass,
                        replica_groups=[list(range(NC))],
                        ins=[fin[t][:]], outs=[featf[t][:]],
                    )
                for t in range(T):
                    aggs = agg_d[t % 2]
                    edge_phase(t, featf[t][:], aggs)
                    dense_phase(t, 1, aggs)
                    nc.gpsimd.collective_compute(
                        "AllGather", ALU.bypass,
                        replica_groups=[list(range(NC))],
                        ins=[ag_in[t][:]], outs=[h1f[t][:]],
                    )
                for t in range(T):
                    aggs = agg_d[t % 2]
                    edge_phase(t, h1f[t][:], aggs)
                    dense_phase(t, 2, aggs)

            # ---- GRU + MLP, feature-major chunks of 512 nodes
            with (
                tc.tile_pool(name="gwork", bufs=2) as pool,
                tc.tile_pool(name="gps", bufs=1, space="PSUM") as psum,
            ):
                lrow = pool.tile([1, NSP], f32, tag="lrow")
                for m in range(MTILES):
                    cols = slice(512 * m, 512 * (m + 1))
                    hA = pool.tile([D, 512], f32, tag="hA")
                    hB = pool.tile([D, 512], f32, tag="hB")
                    nc.vector.memset(hA[:], 0.0)
                    for t in range(T):
                        hin = hA if t % 2 == 0 else hB
                        hout = hB if t % 2 == 0 else hA
                        xT = pool.tile([D, 512], f32, tag="xT")
                        nc.sync.dma_start(xT[:], h2T_d[t][:, cols])
                        ps_r = psum.tile([D, 512], f32, tag="ps_r", space="PSUM")
                        ps_z = psum.tile([D, 512], f32, tag="ps_z", space="PSUM")
                        ps_n = psum.tile([D, 512], f32, tag="ps_n", space="PSUM")
                        ps_h = psum.tile([D, 512], f32, tag="ps_h", space="PSUM")
                        nc.tensor.matmul(ps_r[:], wt["wih"][:, 0:D], xT[:],
                                         start=True, stop=False)
                        nc.tensor.matmul(ps_r[:], wt["whh"][:, 0:D], hin[:],
                                         start=False, stop=True)
                        nc.tensor.matmul(ps_z[:], wt["wih"][:, D:2*D], xT[:],
                                         start=True, stop=False)
                        nc.tensor.matmul(ps_z[:], wt["whh"][:, D:2*D], hin[:],
                                         start=False, stop=True)
                        nc.tensor.matmul(ps_n[:], wt["wih"][:, 2*D:3*D], xT[:],
                                         start=True, stop=True)
                        nc.tensor.matmul(ps_h[:], wt["whh"][:, 2*D:3*D], hin[:],
                                         start=True, stop=True)
                        r_sb = pool.tile([D, 512], f32, tag="r_sb")
                        z_sb = pool.tile([D, 512], f32, tag="z_sb")
                        hn_sb = pool.tile([D, 512], f32, tag="hn_sb")
                        n_sb = pool.tile([D, 512], f32, tag="n_sb")
                        nc.scalar.activation(r_sb[:], ps_r[:], AF.Sigmoid,
                                             bias=wt["gbr"][:])
                        nc.scalar.activation(z_sb[:], ps_z[:], AF.Sigmoid,
                                             bias=wt["gbz"][:])
                        nc.scalar.activation(hn_sb[:], ps_h[:], AF.Identity,
                                             bias=wt["gbhn"][:])
                        nc.vector.tensor_tensor(out=hn_sb[:], in0=r_sb[:],
                                                in1=hn_sb[:], op=ALU.mult)
                        nc.vector.tensor_tensor(out=hn_sb[:], in0=ps_n[:],
                                                in1=hn_sb[:], op=ALU.add)
                        nc.scalar.activation(n_sb[:], hn_sb[:], AF.Tanh,
                                             bias=wt["gbin"][:])
                        # h' = n + z*(h - n)
                        nc.vector.tensor_tensor(out=hout[:], in0=hin[:],
                                                in1=n_sb[:], op=ALU.subtract)
                        nc.vector.tensor_tensor(out=hout[:], in0=z_sb[:],
                                                in1=hout[:], op=ALU.mult)
                        nc.vector.tensor_tensor(out=hout[:], in0=n_sb[:],
                                                in1=hout[:], op=ALU.add)
                    hlast = hA if T % 2 == 0 else hB
                    ps_f = psum.tile([D, 512], f32, tag="ps_f", space="PSUM")
                    nc.tensor.matmul(ps_f[:], wt["wc1"][:], hlast[:],
                                     start=True, stop=True)
                    zf = pool.tile([D, 512], f32, tag="zf")
                    nc.scalar.activation(zf[:], ps_f[:], AF.Relu,
                                         bias=wt["bc1c"][:])
                    ps_l = psum.tile([1, 512], f32, tag="ps_l", space="PSUM")
                    nc.tensor.matmul(ps_l[:], wt["wc2"][:], zf[:],
                                     start=True, stop=True)
                    nc.scalar.activation(lrow[:, cols], ps_l[:], AF.Identity,
                                         bias=wt["bc2c"][:])
                nc.sync.dma_start(out_d[:], lrow[:, 0:NS])

    nc.compile()
    return nc


# --------------------------------------------------------------------------
# entry point
# --------------------------------------------------------------------------

def kernel(**inputs):
    cfg = CFG
    NC = cfg["NCORES"]
    T, N = cfg["T"], cfg["N"]
    NS = N // NC

    t0 = _time.monotonic()
    per_core, meta = preprocess(inputs["src"], inputs["dst"], inputs["ew"], cfg)
    _log(f"preprocess: {_time.monotonic() - t0:.1f}s")
    t1 = _time.monotonic()
    feat = np.asarray(inputs["feat"], np.float32)
    wts = make_weight_inputs(
        inputs["W1"], inputs["b1"], inputs["W2"], inputs["b2"],
        inputs["Wih"], inputs["Whh"], inputs["bih"], inputs["bhh"],
        inputs["Wc1"], inputs["bc1"], inputs["Wc2"], inputs["bc2"], cfg)
    in_maps = pack_inputs(per_core, meta, feat, wts, cfg)
    _log(f"pack: {_time.monotonic() - t1:.1f}s")
    t2 = _time.monotonic()
    nc = build_program(meta)
    _log(f"build+bass-compile: {_time.monotonic() - t2:.1f}s")

    from concourse.bass_utils import run_bass_kernel_spmd
    kwargs = {}
    if TRACE:
        kwargs = dict(trace=True, trace_cores=list(range(NC)))
    t3 = _time.monotonic()
    try:
        res = run_bass_kernel_spmd(nc, in_maps, list(range(NC)), **kwargs)
    except (ImportError, ModuleNotFoundError):
        # NTFF profiling hook unavailable in this environment
        res = run_bass_kernel_spmd(nc, in_maps, list(range(NC)))
    wall_ns = (_time.monotonic() - t3) * 1e9
    _log(f"run_bass_kernel_spmd: {wall_ns / 1e9:.1f}s")
    global LAST_EXEC_NS
    LAST_EXEC_NS = res.exec_time_ns if res.exec_time_ns else int(wall_ns)
    out = np.concatenate(
        [np.asarray(res.results[c]["out"]).reshape(NS) for c in range(NC)])
    return out.astype(np.float32)


if __name__ == "__main__":
    pass
